# revision 1
# baseline (speedup 1.0000x reference)
"""MultiHeadAxialAttention TRN2 kernel.

Problem: x[4,128,128,512] -> 1x1 conv q/k/v projections -> axial attention
(column attention over H, then row attention over W, per head) -> [4,128,128,512].

Sharding: core = (batch b, head-group of 4 heads); 8 cores, zero cross-core
communication. Host pre-transposes x[b] to x^T [512, 16384] so the device
never transposes x; host reassembles the [n, w, h, d]-laid-out per-core
outputs into the reference channel order (channel = d*8 + n).

Per-core pipeline (4 heads, processed as 2 groups of 2):
  A. projections: Q^T,K^T,V^T = W^T @ x^T in fp32r (N=512 -> full PE rate),
     outputs cast to bf16.  V^T is PE-transposed per column into V_sb[h,w,d|1]
     (with a ones column for softmax denominators).
  B. per head: column attention (scores S^T[g,h] per w, exp on ScalarE with
     scale=1/8, clip after exp with exp(+-clip/8) bounds on VectorE, then
     xv[h,d]+rowsum via one matmul against [V_w | 1]); xv planes PE-transposed
     to [w,h,d] with softmax normalization folded into the transpose copy via
     a broadcast multiply; row attention the same way; final normalization
     folded into the psum->sbuf output copy; DMA out per 4-row chunk.
"""
import sys
import os
import math

import numpy as np
import ml_dtypes

if "/opt/trn_rl_repo" not in sys.path:
    sys.path.insert(0, "/opt/trn_rl_repo")

B, H, W, C = 4, 128, 128, 512
NH, D = 8, 64
NCORES = 8
HEADS_PER_CORE = 4
NGROUPS = 2          # head groups per core, 2 heads each
PIX = H * W          # 16384, h-major (pix = h*128 + w)
CLIP = 1.0 - 1e-7
SCALE = 1.0 / math.sqrt(D)   # 1/8
EXP_LO = float(np.float32(math.exp(-CLIP * SCALE)))
EXP_HI = float(np.float32(math.exp(CLIP * SCALE)))

_CACHE = {}


def _build_bass():
    import concourse.bacc as bacc
    import concourse.tile as tile
    import concourse.mybir as mybir
    from concourse import masks

    F32 = mybir.dt.float32
    F32R = mybir.dt.float32r
    BF16 = mybir.dt.bfloat16
    Act = mybir.ActivationFunctionType
    Alu = mybir.AluOpType

    nc = bacc.Bacc(None, target_bir_lowering=False)

    # DRAM I/O (per-core shapes; SPMD over in_maps)
    xT_d = nc.dram_tensor("xT", [4, 128, PIX], BF16, kind="ExternalInput")
    wq_d = nc.dram_tensor("wq", [4, 128, 256], BF16, kind="ExternalInput")
    wk_d = nc.dram_tensor("wk", [4, 128, 256], BF16, kind="ExternalInput")
    wv_d = nc.dram_tensor("wv", [4, 128, 256], BF16, kind="ExternalInput")
    bq_d = nc.dram_tensor("bq", [128, 2], F32, kind="ExternalInput")
    bk_d = nc.dram_tensor("bk", [128, 2], F32, kind="ExternalInput")
    bv_d = nc.dram_tensor("bv", [128, 2], F32, kind="ExternalInput")
    out_d = nc.dram_tensor("out", [4, W, H, D], F32, kind="ExternalOutput")

    with tile.TileContext(nc) as tc:
        with (
            tc.tile_pool(name="const", bufs=1) as constp,
            tc.tile_pool(name="persist", bufs=1) as persist,
            tc.tile_pool(name="xt", bufs=3) as xtp,
            tc.tile_pool(name="ebuf", bufs=3) as ebufp,
            tc.tile_pool(name="obuf", bufs=3) as obufp,
            tc.tile_pool(name="ps", bufs=2, space="PSUM") as psp,
            tc.tile_pool(name="ps_small", bufs=2, space="PSUM") as psps,
        ):
            ident_f32 = constp.tile([128, 128], F32, tag="id32")
            ident_bf16 = constp.tile([128, 128], BF16, tag="id16")
            masks.make_identity(nc, ident_f32[:])
            masks.make_identity(nc, ident_bf16[:])

            wsb = {}
            bsb = {}
            for nm, wd, bd in (("q", wq_d, bq_d), ("k", wk_d, bk_d),
                               ("v", wv_d, bv_d)):
                wt = constp.tile([128, 4, 256], BF16, tag=f"w{nm}")
                for kc in range(4):
                    nc.sync.dma_start(wt[:, kc, :], wd[kc])
                bt = constp.tile([128, 2], F32, tag=f"b{nm}")
                nc.sync.dma_start(bt[:], bd[:])
                wsb[nm] = wt
                bsb[nm] = bt

            # persistent per-group tensors
            QT = persist.tile([128, PIX], BF16, tag="QT")
            KT = persist.tile([128, PIX], BF16, tag="KT")
            VTh = persist.tile([128, PIX // 2], BF16, tag="VTh")
            V_sb = [persist.tile([128, W, D + 1], BF16, tag=f"V{j}",
                                 name=f"V{j}") for j in range(2)]
            xv_sb = persist.tile([128, W, D], BF16, tag="xv")
            xv2 = persist.tile([128, H, D + 1], BF16, tag="xv2")
            sums = persist.tile([128, W], F32, tag="sums")
            rvt = persist.tile([128, H], F32, tag="rvt")

            NT = PIX // 512   # 32 pixel tiles of 512

            KREPS = int(os.environ.get("KREPS", "1"))
            for rep in range(KREPS):
              for g in range(NGROUPS):
                  fsl = slice(g * 128, (g + 1) * 128)

                  # ---- phase A: projections (two halves so VTh stays small) ----
                  for half in range(2):
                      for t in range(NT // 2):
                          tt = half * (NT // 2) + t
                          xt = xtp.tile([128, 4, 512], BF16, tag="xt")
                          nc.sync.dma_start(
                              xt[:],
                              xT_d[:, :, tt * 512:(tt + 1) * 512].transpose(
                                  [1, 0, 2]))
                          for nm, dst, off in (("q", QT, tt * 512),
                                               ("k", KT, tt * 512),
                                               ("v", VTh, t * 512)):
                              ps = psp.tile([128, 512], F32, tag="big_ps")
                              for kc in range(4):
                                  nc.tensor.matmul(
                                      ps[:], wsb[nm][:, kc, fsl], xt[:, kc, :],
                                      start=(kc == 0), stop=(kc == 3))
                              dslice = dst[:, off:off + 512]
                              if nm == "k":
                                  nc.vector.tensor_scalar(
                                      dslice, ps[:], bsb[nm][:, g:g + 1], None,
                                      Alu.add)
                              else:
                                  nc.scalar.activation(
                                      dslice, ps[:], Act.Identity,
                                      bias=bsb[nm][:, g:g + 1], scale=1.0)

                      # V rearrange for this half: V^T[f, pix] -> V_sb[h, w, d]
                      h0 = half * 64
                      for wb in range(16):
                          tps = psp.tile([128, 8, 128], BF16, tag="big_ps")
                          for j in range(8):
                              w = wb * 8 + j
                              nc.tensor.transpose(
                                  tps[h0:h0 + 64, j, :],
                                  VTh[:, w::128], ident_bf16[:])
                          for jh in range(2):
                              nc.vector.tensor_copy(
                                  V_sb[jh][h0:h0 + 64, wb * 8:wb * 8 + 8, 0:D],
                                  tps[h0:h0 + 64, :, jh * 64:(jh + 1) * 64])
                  for jh in range(2):
                      nc.vector.memset(V_sb[jh][:, :, D], 1.0)

                  # ---- phase B: attention per head ----
                  KPHASE = int(os.environ.get("KPHASE", "40"))
                  def dbg_out(n_loc_, src_bf16):
                      for hf in range(2):
                          dbg = obufp.tile([128, W, D // 2], F32, tag="dbg",
                                           name="dbg", bufs=1)
                          nc.vector.tensor_copy(
                              dbg[:], src_bf16[:, :, hf * 32:(hf + 1) * 32])
                          nc.sync.dma_start(
                              out_d[n_loc_, :, :, hf * 32:(hf + 1) * 32], dbg[:])
                  if KPHASE < 20:
                      for jh in range(2):
                          dbg_out(g * 2 + jh, V_sb[jh][:, :, 0:D])
                      continue
                  for jh in range(2):
                      n_loc = g * 2 + jh
                      hsl = slice(jh * 64, (jh + 1) * 64)

                      # column attention (contract over H, per column w)
                      for wb in range(16):
                          sps = psp.tile([128, 8, 128], F32, tag="big_ps")
                          for j in range(8):
                              w = wb * 8 + j
                              nc.tensor.matmul(
                                  sps[:, j, :], KT[hsl, w::128], QT[hsl, w::128],
                                  start=True, stop=True)
                          ex = ebufp.tile([128, 8, 128], BF16, tag="ex")
                          nc.scalar.activation(ex[:], sps[:], Act.Exp,
                                               scale=SCALE)
                          nc.vector.tensor_scalar(ex[:], ex[:], EXP_LO, EXP_HI,
                                                  Alu.max, Alu.min)
                          xvps = psps.tile([128, 2, 4, 128], F32, tag="small_ps")
                          for j in range(8):
                              w = wb * 8 + j
                              nc.tensor.matmul(
                                  xvps[:, j // 4, j % 4, 0:D + 1],
                                  ex[:, j, :], V_sb[jh][:, w, :],
                                  start=True, stop=True)
                          nc.scalar.copy(
                              xv_sb[:, wb * 8:wb * 8 + 8, :].rearrange(
                                  "p (a b) d -> p a b d", a=2),
                              xvps[:, :, :, 0:D])
                          nc.scalar.copy(
                              sums[:, wb * 8:wb * 8 + 8].rearrange(
                                  "p (a b) -> p a b", a=2),
                              xvps[:, :, :, D])

                      if KPHASE < 30:
                          dbg_out(n_loc, xv_sb[:])
                          continue

                      # transpose xv [h,w,d] -> xv2 [w,h,d], normalizing by
                      # 1/s_v (broadcast multiply) during the psum->sbuf copy
                      stp = psp.tile([128, 128], F32, tag="big_ps")
                      nc.tensor.transpose(stp[:], sums[:], ident_f32[:])
                      nc.vector.reciprocal(rvt[:], stp[:])
                      for db in range(16):
                          mps = psp.tile([128, 4, 128], BF16, tag="big_ps")
                          for j in range(4):
                              d = db * 4 + j
                              nc.tensor.transpose(
                                  mps[:, j, :], xv_sb[:, :, d], ident_bf16[:])
                          nc.vector.tensor_tensor(
                              xv2[:, :, db * 4:db * 4 + 4].transpose([0, 2, 1]),
                              mps[:],
                              rvt[:].unsqueeze(1).broadcast_to([128, 4, 128]),
                              Alu.mult)
                      nc.vector.memset(xv2[:, :, D], 1.0)

                      if KPHASE < 35:
                          dbg_out(n_loc, xv2[:, :, 0:D])
                          continue

                      # row attention (contract over W, per row h)
                      for hb in range(16):
                          sps2 = psp.tile([128, 8, 128], F32, tag="big_ps")
                          for j in range(8):
                              h = hb * 8 + j
                              nc.tensor.matmul(
                                  sps2[:, j, :],
                                  KT[hsl, h * 128:(h + 1) * 128],
                                  QT[hsl, h * 128:(h + 1) * 128],
                                  start=True, stop=True)
                          eu = ebufp.tile([128, 8, 128], BF16, tag="ex")
                          nc.scalar.activation(eu[:], sps2[:], Act.Exp,
                                               scale=SCALE)
                          nc.vector.tensor_scalar(eu[:], eu[:], EXP_LO, EXP_HI,
                                                  Alu.max, Alu.min)
                          xups = psps.tile([128, 2, 4, 128], F32, tag="small_ps")
                          for j in range(8):
                              h = hb * 8 + j
                              nc.tensor.matmul(
                                  xups[:, j // 4, j % 4, 0:D + 1],
                                  eu[:, j, :], xv2[:, h, :],
                                  start=True, stop=True)
                          ob = obufp.tile([128, 2, 4, D], F32, tag="ob")
                          if KPHASE < 40:
                              nc.vector.tensor_copy(ob[:], xups[:, :, :, 0:D])
                          else:
                              ru = obufp.tile([128, 2, 4], F32, tag="ru")
                              nc.vector.reciprocal(ru[:], xups[:, :, :, D])
                              nc.vector.tensor_tensor(
                                  ob[:], xups[:, :, :, 0:D],
                                  ru[:].unsqueeze(3).broadcast_to([128, 2, 4, D]),
                                  Alu.mult)
                          nc.sync.dma_start(
                              out_d[n_loc, :, hb * 8:hb * 8 + 8, :].rearrange(
                                  "p (a b) d -> p a b d", a=2), ob[:])

    nc.compile()
    return nc


def _get_nc():
    if "nc" not in _CACHE:
        _CACHE["nc"] = _build_bass()
    return _CACHE["nc"]


def kernel(x, wq, bq, wk, bk, wv, bv):
    from concourse.bass_utils import run_bass_kernel_spmd

    x = np.asarray(x, dtype=np.float32)
    wq = np.asarray(wq, dtype=np.float32)
    wk = np.asarray(wk, dtype=np.float32)
    wv = np.asarray(wv, dtype=np.float32)
    bq = np.asarray(bq, dtype=np.float32)
    bk = np.asarray(bk, dtype=np.float32)
    bv = np.asarray(bv, dtype=np.float32)

    nc = _get_nc()

    in_maps = []
    for core in range(NCORES):
        b = core // 2
        g2 = core % 2
        heads = list(range(g2 * 4, g2 * 4 + 4))
        cols = np.concatenate(
            [np.arange(n * D, (n + 1) * D) for n in heads])
        xb = x[b].reshape(PIX, C)
        xT = np.ascontiguousarray(xb.T).reshape(4, 128, PIX)
        in_maps.append({
            "xT": xT.astype(ml_dtypes.bfloat16),
            "wq": np.ascontiguousarray(wq[:, cols]).reshape(
                4, 128, 256).astype(ml_dtypes.bfloat16),
            "wk": np.ascontiguousarray(wk[:, cols]).reshape(
                4, 128, 256).astype(ml_dtypes.bfloat16),
            "wv": np.ascontiguousarray(wv[:, cols]).reshape(
                4, 128, 256).astype(ml_dtypes.bfloat16),
            "bq": np.ascontiguousarray(bq[cols].reshape(2, 128).T),
            "bk": np.ascontiguousarray(bk[cols].reshape(2, 128).T),
            "bv": np.ascontiguousarray(bv[cols].reshape(2, 128).T),
        })

    res = run_bass_kernel_spmd(nc, in_maps, list(range(NCORES)),
                               trace=bool(os.environ.get("KTRACE")))
    _CACHE["last_results"] = res

    out = np.empty((B, H, W, C), dtype=np.float32)
    for core in range(NCORES):
        r = np.asarray(res.results[core]["out"], dtype=np.float32)
        b = core // 2
        g2 = core % 2
        for jn, n in enumerate(range(g2 * 4, g2 * 4 + 4)):
            # r[jn] is [w, h, d]; reference channel order is d*NH + n
            out[b, :, :, n::NH] = r[jn].transpose(1, 0, 2)
    return out

